# revision 29
# baseline (speedup 1.0000x reference)
"""Trainium2 Bass kernel for batched multi-head attention with additive mask.

Problem (full shapes): q,k,v [2,16,2048,64] f32, mask [1,1,2048,2048] f32,
scale scalar; out = softmax(q@k^T/scale + mask) @ v -> [2,16,2048,64].

Sharding: B*H = 32 heads split over 8 cores (4 heads/core), pure data
parallel, no collectives. The shared mask is replicated to every core.

Key idea vs the straightforward version: softmax(S + M) uses
exp(S + M) = exp(S) * exp(M), and M is shared across all heads. The host
precomputes expM = exp(M^T) once (bf16); the device then needs only
  P = exp(S) (ScalarE, PSUM->SBUF bf16)  *  expM tile (DVE bf16 2x mul)
instead of an f32 PSUM mask-add (1x DVE) + exp. The device also skips
normalization entirely: it emits O' = [denom | O]^T = [ones|V]'^T @ P^T
per head ([65, SQ] f32, denominator in row 0) and the host divides and
transposes. That removes the PE transposes, ScalarE copies and DVE
reciprocals of the normalize stage and frees PSUM banks.

Per-core device algorithm (per head pair, q-half, kv-tile):
  - S^T [128 kv, 1024 q] = kT.T @ qT, contraction d=64. The two heads of
    a pair sit in partitions 0-63 / 64-127 of pair-stacked qT/kT tiles,
    so their matmuls auto-derive PE tile_position (0,0)/(64,0) and run
    concurrently in the array (row tiling), recovering the half-array
    loss of the d=64 contraction.
  - P0 = exp(S^T): ScalarE PSUM -> SBUF bf16.
  - P = P0 * expM[t]: DVE bf16 tensor_tensor (2x mode).
  - O'^T [65, 1024] += V'[kv,65].T @ P^T, V' = [ones | V] bf16,
    accumulated f32 in PSUM over the 16 kv tiles.
  - DVE copy O' PSUM -> SBUF, DMA to DRAM [head, 65, SQ].
PE warm-up matmuls on a zero tile (no DMA dependency) keep the HAM clock
gate at 8/8 through the input-DMA prologue.
"""

import numpy as np

B, H, SQ, SKV, D = 2, 16, 2048, 2048, 64
NCORES = 8
HPC = (B * H) // NCORES  # heads per core = 4
NPAIR = HPC // 2  # head pairs per core = 2
KT = SKV // 128  # kv tiles = 16
QHALF = SQ // 2  # 1024
DC = D + 1  # 65 rows: [denom | O]

_cached = {}


def _build():
    from concourse import bacc
    import concourse.mybir as mybir
    import concourse.tile as tile

    F32 = mybir.dt.float32
    BF16 = mybir.dt.bfloat16
    EXP = mybir.ActivationFunctionType.Exp

    nc = bacc.Bacc("TRN2", target_bir_lowering=False, debug=False,
                   num_devices=NCORES)

    qT = nc.declare_dram_parameter("qT", [NPAIR, 128, SQ], BF16, isOutput=False)
    kT = nc.declare_dram_parameter("kT", [NPAIR, 128, SKV], BF16, isOutput=False)
    # vA is host-pre-arranged to the SBUF-resident layout [128, KT*DC] so
    # its DMA is fully contiguous (the naive [SKV, DC] layout needs 130-byte
    # strided segments whose descriptors occupy the DMA queue for ~12us and
    # starve the mask-tile loads behind them)
    vA = nc.declare_dram_parameter("vA", [HPC, 128, KT * DC], BF16, isOutput=False)
    expM = nc.declare_dram_parameter("expM", [SKV, SQ], BF16, isOutput=False)
    out = nc.declare_dram_parameter("out", [HPC, DC, SQ], F32, isOutput=True)

    with tile.TileContext(nc) as tc:
        with (
            tc.tile_pool(name="qk", bufs=1) as qk_pool,
            tc.tile_pool(name="vp", bufs=1) as v_pool,
            tc.tile_pool(name="m", bufs=1) as m_pool,
            tc.tile_pool(name="z", bufs=1) as z_pool,
            tc.tile_pool(name="p", bufs=2) as p_pool,
            tc.tile_pool(name="pm", bufs=2) as pm_pool,
            tc.tile_pool(name="osb", bufs=4) as osb_pool,
            tc.tile_pool(name="ps_s", bufs=1, space="PSUM") as ps_s,
            tc.tile_pool(name="ps_o", bufs=1, space="PSUM") as ps_o,
        ):
            # zero tile for PE warm-up: no DMA dependency, issues at t=0
            zz = z_pool.tile([128, 640], BF16, tag="z", name="zz")
            nc.vector.memset(zz[:], 0.0)

            # resident qT/kT, pair-stacked [128, seq]; pair 0 first so the
            # main loop can start as early as possible
            qT_sb = [None] * NPAIR
            kT_sb = [None] * NPAIR
            qt = qk_pool.tile([128, SQ], BF16, tag="q0", name="q0")
            nc.sync.dma_start(qt[:], qT[0])
            qT_sb[0] = qt
            kt = qk_pool.tile([128, SKV], BF16, tag="k0", name="k0")
            nc.scalar.dma_start(kt[:], kT[0])
            kT_sb[0] = kt

            # PE warm-up part 1: back-to-back matmuls on the zero tile keep
            # the PE busy while the first input DMAs land.
            wu_ps = ps_s.tile([128, QHALF], F32, tag="sA", name="wu")
            for w in range(16):
                nc.tensor.matmul(
                    wu_ps[:, :512], zz[:, :128], zz[:, 128:640],
                    start=True, stop=True,
                )

            # Input DMAs are spread across engine DGE queues (each engine
            # owns one HW queue, ~180 GB/s each): q/k on sync, V + late
            # masks on gpsimd (idle all kernel), early masks on vector,
            # last masks on scalar. One serial queue would take ~20us+
            # before the first O matmul can run.
            # Each mask tile is DMA'd in two column halves: the first
            # q-half of compute reads only columns 0:1024, so the hi
            # halves (4.2MB) are deferred off the ramp's critical path
            # (subtile deps let each mul wait on just its half).
            m_sb = [None] * KT

            def m_dma(t, eng, half):
                if m_sb[t] is None:
                    m_sb[t] = m_pool.tile(
                        [128, SQ], BF16, tag=f"m{t}", name=f"m{t}"
                    )
                c = half * QHALF
                eng.dma_start(
                    m_sb[t][:, c:c + QHALF],
                    expM[t * 128:(t + 1) * 128, c:c + QHALF],
                )

            for t in range(0, 8, 2):
                m_dma(t, nc.scalar, 0)
                m_dma(t + 1, nc.sync, 0)

            v_sb = []
            for h in range(HPC):
                vt = v_pool.tile([128, KT * DC], BF16, tag=f"v{h}", name=f"v{h}")
                nc.gpsimd.dma_start(vt[:], vA[h])
                v_sb.append(vt)

            # PE warm-up part 2: gated on the k0 DMA, so it runs gap-free
            # right before the first real S matmuls. The HAM clock gate
            # only reaches 8/8 after ~3.4us of *continuous* PE busy; the
            # main loop's dependency micro-gaps never re-warm it, so the
            # warm state must be established here and never dropped.
            wu2_ps = ps_s.tile([128, QHALF], F32, tag="sB", name="wu2")
            for w in range(8):
                nc.tensor.matmul(
                    wu2_ps[:, :512], zz[:, :128], kT_sb[0][:, :512],
                    start=True, stop=True,
                )

            for t in range(8, KT):
                m_dma(t, nc.gpsimd, 0)
            # hi mask halves: not needed until the second q-half (~50us)
            for t in range(KT):
                m_dma(t, (nc.sync, nc.scalar, nc.gpsimd)[t % 3], 1)

            # pair-1 q/k: not needed until ~90us in
            qt = qk_pool.tile([128, SQ], BF16, tag="q1", name="q1")
            nc.gpsimd.dma_start(qt[:], qT[1])
            qT_sb[1] = qt
            kt = qk_pool.tile([128, SKV], BF16, tag="k1", name="k1")
            nc.gpsimd.dma_start(kt[:], kT[1])
            kT_sb[1] = kt

            for pr in range(NPAIR):
                heads = (("A", 0, 2 * pr), ("B", 64, 2 * pr + 1))
                for half in range(2):
                    q0 = half * QHALF
                    o_ps = {}
                    for sub, _, _ in heads:
                        for c2 in range(2):
                            o_ps[(sub, c2)] = ps_o.tile(
                                [DC, 512], F32, tag=f"o{sub}{c2}",
                                name=f"o{sub}{c2}",
                            )
                    # zero-weight filler matmuls: accumulate +0 into the
                    # live O tiles (start=False -> pure PE busy-work, no
                    # extra PSUM). They pad the PE stream just enough that
                    # the HAM clock gate sees near-continuous activity and
                    # stays 8/8; without them HAM re-throttles the whole
                    # kernel to 1.2 GHz.
                    def filler(fc, n):
                        nc.tensor.matmul(
                            o_ps[(("A", "B")[fc % 2], fc // 2)][:, :n],
                            zz[:, :DC],
                            m_sb[0][:, :n],
                            start=False,
                            stop=False,
                        )

                    def emit_o(t):
                        # O(t) is emitted one slot late so the in-order PE
                        # queue never stalls on mul(t): by the time the PE
                        # reaches O(t), its P tile has long been ready.
                        for sub, _, h in heads:
                            for c2 in range(2):
                                nc.tensor.matmul(
                                    o_ps[(sub, c2)][:],
                                    v_sb[h][:, t * DC:(t + 1) * DC],
                                    p_t[t % 2][sub][:, c2 * 512:(c2 + 1) * 512],
                                    start=(t == 0),
                                    stop=(t == KT - 1),
                                )

                    p_t = [{}, {}]
                    for t in range(KT):
                        # O(t-1) first: it is dependency-free by now, runs
                        # while exp(t-1) still executes, and leaves nothing
                        # between S_B(t) and S_A(t+1) to delay the critical
                        # exp -> S -> exp chain.
                        if t >= 1:
                            emit_o(t - 1)
                        for sub, r0, _ in heads:
                            # two heads' S matmuls occupy PE row groups
                            # 0-63 / 64-127 (auto tile_position)
                            s_ps = ps_s.tile([128, QHALF], F32, tag=f"s{sub}")
                            for c2 in range(2):
                                nc.tensor.matmul(
                                    s_ps[:, c2 * 512:(c2 + 1) * 512],
                                    kT_sb[pr][r0:r0 + 64, t * 128:(t + 1) * 128],
                                    qT_sb[pr][r0:r0 + 64,
                                              q0 + c2 * 512:q0 + (c2 + 1) * 512],
                                    start=True,
                                    stop=True,
                                )
                            p0 = p_pool.tile([128, QHALF], BF16, tag=f"p{sub}")
                            nc.scalar.activation(p0[:], s_ps[:], EXP)
                            pm = pm_pool.tile([128, QHALF], BF16, tag=f"pm{sub}")
                            nc.vector.tensor_mul(
                                out=pm[:], in0=p0[:],
                                in1=m_sb[t][:, q0:q0 + QHALF],
                            )
                            p_t[t % 2][sub] = pm
                        if 1 <= t < KT - 1:
                            filler(t % 4, 256)
                            filler((t + 1) % 4, 256)
                        if pr == 0 and half == 0 and t <= 5:
                            # extra ramp fillers: the first slots run at
                            # input-DMA pace with the PE queue blocked, and
                            # a >3.4us PE lull here re-throttles the clock
                            # gate for the whole kernel
                            for f in range(3):
                                filler((t + f) % 4, 512)
                    emit_o(KT - 1)
                    # drain: copies split across ScalarE/VectorE so each
                    # O bank frees in ~0.7us and the next half's first O
                    # matmuls (WAR on these banks) never stall the PE queue
                    for sub, _, h in heads:
                        o_sb = osb_pool.tile([DC, QHALF], F32, tag="osb")
                        oeng = nc.sync if sub == "A" else nc.gpsimd
                        nc.scalar.copy(o_sb[:, 0:512], o_ps[(sub, 0)][:])
                        oeng.dma_start(out[h, :, q0:q0 + 512], o_sb[:, 0:512])
                        nc.vector.tensor_copy(o_sb[:, 512:1024], o_ps[(sub, 1)][:])
                        oeng.dma_start(out[h, :, q0 + 512:q0 + QHALF],
                                       o_sb[:, 512:1024])
    nc.compile()
    return nc


def _prep_in_maps(q, k, v, mask, s):
    import ml_dtypes

    bf16 = ml_dtypes.bfloat16
    # host prep: fold 1/scale into q; transpose to [d, seq]; pair-stack heads
    qh = (q / s).reshape(B * H, SQ, D).transpose(0, 2, 1)  # [32, 64, 2048]
    kh = k.reshape(B * H, SKV, D).transpose(0, 2, 1)
    vh = v.reshape(B * H, SKV, D)
    vA = np.concatenate(
        [np.ones((B * H, SKV, 1), dtype=np.float32), vh], axis=2
    ).astype(bf16)  # [32, 2048, 65], col 0 = ones
    # pre-arrange to the SBUF layout [head, 128, KT*DC] for contiguous DMA
    vA = np.ascontiguousarray(
        vA.reshape(B * H, KT, 128, DC).transpose(0, 2, 1, 3)
    ).reshape(B * H, 128, KT * DC)
    # exp of the transposed mask, shared across heads: exp(S+M) = exp(S)*exp(M)
    expM = np.exp(np.ascontiguousarray(mask.reshape(SQ, SKV).T)).astype(bf16)

    in_maps = []
    for c in range(NCORES):
        h0 = c * HPC
        qTc = np.ascontiguousarray(
            qh[h0:h0 + HPC].reshape(NPAIR, 128, SQ)
        ).astype(bf16)
        kTc = np.ascontiguousarray(
            kh[h0:h0 + HPC].reshape(NPAIR, 128, SKV)
        ).astype(bf16)
        vAc = np.ascontiguousarray(vA[h0:h0 + HPC])
        in_maps.append({"qT": qTc, "kT": kTc, "vA": vAc, "expM": expM})
    return in_maps


def kernel(q, k, v, mask, scale):
    from concourse.bass_utils import run_bass_kernel_spmd

    q = np.asarray(q, dtype=np.float32)
    k = np.asarray(k, dtype=np.float32)
    v = np.asarray(v, dtype=np.float32)
    mask = np.asarray(mask, dtype=np.float32)
    s = float(np.asarray(scale))

    in_maps = _prep_in_maps(q, k, v, mask, s)

    if "nc" not in _cached:
        _cached["nc"] = _build()
    res = run_bass_kernel_spmd(_cached["nc"], in_maps, list(range(NCORES)))

    # device emits unnormalized [head, 65, SQ]: row 0 = softmax denominator
    outs = []
    for c in range(NCORES):
        o = res.results[c]["out"]  # [HPC, DC, SQ] f32
        outs.append(o[:, 1:, :] / o[:, 0:1, :])
    full = np.concatenate(outs, axis=0)  # [32, 64, SQ]
    return np.ascontiguousarray(full.transpose(0, 2, 1)).reshape(B, H, SQ, D)


# revision 31
# speedup vs baseline: 1.0872x; 1.0872x over previous
"""Trainium2 Bass kernel for batched multi-head attention with additive mask.

Problem (full shapes): q,k,v [2,16,2048,64] f32, mask [1,1,2048,2048] f32,
scale scalar; out = softmax(q@k^T/scale + mask) @ v -> [2,16,2048,64].

Sharding: B*H = 32 heads split over 8 cores (4 heads/core), pure data
parallel, no collectives. The shared mask is replicated to every core.

Key idea vs the straightforward version: softmax(S + M) uses
exp(S + M) = exp(S) * exp(M), and M is shared across all heads. The host
precomputes expM = exp(M^T) once (bf16); the device then needs only
  P = exp(S) (ScalarE, PSUM->SBUF bf16)  *  expM tile (DVE bf16 2x mul)
instead of an f32 PSUM mask-add (1x DVE) + exp. The device also skips
normalization entirely: it emits O' = [denom | O]^T = [ones|V]'^T @ P^T
per head ([65, SQ] f32, denominator in row 0) and the host divides and
transposes. That removes the PE transposes, ScalarE copies and DVE
reciprocals of the normalize stage and frees PSUM banks.

Per-core device algorithm (per head pair, q-half, kv-tile):
  - S^T [128 kv, 1024 q] = kT.T @ qT, contraction d=64. The two heads of
    a pair sit in partitions 0-63 / 64-127 of pair-stacked qT/kT tiles,
    so their matmuls auto-derive PE tile_position (0,0)/(64,0) and run
    concurrently in the array (row tiling), recovering the half-array
    loss of the d=64 contraction.
  - P0 = exp(S^T): ScalarE PSUM -> SBUF bf16.
  - P = P0 * expM[t]: DVE bf16 tensor_tensor (2x mode).
  - O'^T [65, 1024] += V'[kv,65].T @ P^T, V' = [ones | V] bf16,
    accumulated f32 in PSUM over the 16 kv tiles.
  - DVE copy O' PSUM -> SBUF, DMA to DRAM [head, 65, SQ].
PE warm-up matmuls on a zero tile (no DMA dependency) keep the HAM clock
gate at 8/8 through the input-DMA prologue.
"""

import numpy as np

B, H, SQ, SKV, D = 2, 16, 2048, 2048, 64
NCORES = 8
HPC = (B * H) // NCORES  # heads per core = 4
NPAIR = HPC // 2  # head pairs per core = 2
KT = SKV // 128  # kv tiles = 16
QHALF = SQ // 2  # 1024
DC = D + 1  # 65 rows: [denom | O]

_cached = {}


def _build():
    from concourse import bacc
    import concourse.mybir as mybir
    import concourse.tile as tile

    F32 = mybir.dt.float32
    BF16 = mybir.dt.bfloat16
    EXP = mybir.ActivationFunctionType.Exp

    nc = bacc.Bacc("TRN2", target_bir_lowering=False, debug=False,
                   num_devices=NCORES)

    qT = nc.declare_dram_parameter("qT", [NPAIR, 128, SQ], BF16, isOutput=False)
    kT = nc.declare_dram_parameter("kT", [NPAIR, 128, SKV], BF16, isOutput=False)
    # vA is host-pre-arranged to the SBUF-resident layout [128, KT*DC] so
    # its DMA is fully contiguous (the naive [SKV, DC] layout needs 130-byte
    # strided segments whose descriptors occupy the DMA queue for ~12us and
    # starve the mask-tile loads behind them)
    vA = nc.declare_dram_parameter("vA", [HPC, 128, KT * DC], BF16, isOutput=False)
    expM = nc.declare_dram_parameter("expM", [SKV, SQ], BF16, isOutput=False)
    out = nc.declare_dram_parameter("out", [HPC, DC, SQ], F32, isOutput=True)

    with tile.TileContext(nc) as tc:
        with (
            tc.tile_pool(name="qk", bufs=1) as qk_pool,
            tc.tile_pool(name="vp", bufs=1) as v_pool,
            tc.tile_pool(name="m", bufs=1) as m_pool,
            tc.tile_pool(name="z", bufs=1) as z_pool,
            tc.tile_pool(name="p", bufs=2) as p_pool,
            tc.tile_pool(name="pm", bufs=2) as pm_pool,
            tc.tile_pool(name="osb", bufs=4) as osb_pool,
            tc.tile_pool(name="ps_s", bufs=1, space="PSUM") as ps_s,
            tc.tile_pool(name="ps_o", bufs=1, space="PSUM") as ps_o,
        ):
            # zero tile for PE warm-up: no DMA dependency, issues at t=0
            zz = z_pool.tile([128, 640], BF16, tag="z", name="zz")
            nc.vector.memset(zz[:], 0.0)

            # resident qT/kT, pair-stacked [128, seq]; pair 0 first so the
            # main loop can start as early as possible
            qT_sb = [None] * NPAIR
            kT_sb = [None] * NPAIR
            qt = qk_pool.tile([128, SQ], BF16, tag="q0", name="q0")
            nc.sync.dma_start(qt[:], qT[0])
            qT_sb[0] = qt
            kt = qk_pool.tile([128, SKV], BF16, tag="k0", name="k0")
            nc.scalar.dma_start(kt[:], kT[0])
            kT_sb[0] = kt

            # PE warm-up part 1: back-to-back matmuls on the zero tile keep
            # the PE busy while the first input DMAs land.
            wu_ps = ps_s.tile([128, QHALF], F32, tag="sA", name="wu")
            for w in range(16):
                nc.tensor.matmul(
                    wu_ps[:, :512], zz[:, :128], zz[:, 128:640],
                    start=True, stop=True,
                )

            # Input DMAs are spread across engine DGE queues (each engine
            # owns one HW queue, ~180 GB/s each): q/k on sync, V + late
            # masks on gpsimd (idle all kernel), early masks on vector,
            # last masks on scalar. One serial queue would take ~20us+
            # before the first O matmul can run.
            # Each mask tile is DMA'd in two column halves: the first
            # q-half of compute reads only columns 0:1024, so the hi
            # halves (4.2MB) are deferred off the ramp's critical path
            # (subtile deps let each mul wait on just its half).
            m_sb = [None] * KT

            def m_dma(t, eng, half):
                if m_sb[t] is None:
                    m_sb[t] = m_pool.tile(
                        [128, SQ], BF16, tag=f"m{t}", name=f"m{t}"
                    )
                c = half * QHALF
                eng.dma_start(
                    m_sb[t][:, c:c + QHALF],
                    expM[t * 128:(t + 1) * 128, c:c + QHALF],
                )

            # scalar gets only two issues: its FIFO must stay clear for exps
            m_dma(0, nc.scalar, 0)
            m_dma(1, nc.sync, 0)
            m_dma(2, nc.scalar, 0)
            m_dma(3, nc.sync, 0)

            v_sb = []
            for h in range(HPC):
                vt = v_pool.tile([128, KT * DC], BF16, tag=f"v{h}", name=f"v{h}")
                nc.gpsimd.dma_start(vt[:], vA[h])
                v_sb.append(vt)

            # PE warm-up part 2: gated on the k0 DMA, so it runs gap-free
            # right before the first real S matmuls. The HAM clock gate
            # only reaches 8/8 after ~3.4us of *continuous* PE busy; the
            # main loop's dependency micro-gaps never re-warm it, so the
            # warm state must be established here and never dropped.
            wu2_ps = ps_s.tile([128, QHALF], F32, tag="sB", name="wu2")
            for w in range(8):
                nc.tensor.matmul(
                    wu2_ps[:, :512], zz[:, :128], kT_sb[0][:, :512],
                    start=True, stop=True,
                )

            m_dma(4, nc.sync, 0)
            m_dma(5, nc.sync, 0)
            for t in range(6, KT):
                m_dma(t, nc.gpsimd, 0)
            # hi mask halves: not needed until the second q-half (~50us)
            for t in range(KT):
                m_dma(t, nc.sync if t % 2 else nc.gpsimd, 1)

            # pair-1 q/k: not needed until ~90us in
            qt = qk_pool.tile([128, SQ], BF16, tag="q1", name="q1")
            nc.gpsimd.dma_start(qt[:], qT[1])
            qT_sb[1] = qt
            kt = qk_pool.tile([128, SKV], BF16, tag="k1", name="k1")
            nc.gpsimd.dma_start(kt[:], kT[1])
            kT_sb[1] = kt

            for pr in range(NPAIR):
                heads = (("A", 0, 2 * pr), ("B", 64, 2 * pr + 1))
                for half in range(2):
                    q0 = half * QHALF
                    o_ps = {}
                    for sub, _, _ in heads:
                        for c2 in range(2):
                            o_ps[(sub, c2)] = ps_o.tile(
                                [DC, 512], F32, tag=f"o{sub}{c2}",
                                name=f"o{sub}{c2}",
                            )
                    # zero-weight filler matmuls: accumulate +0 into the
                    # live O tiles (start=False -> pure PE busy-work, no
                    # extra PSUM). They pad the PE stream just enough that
                    # the HAM clock gate sees near-continuous activity and
                    # stays 8/8; without them HAM re-throttles the whole
                    # kernel to 1.2 GHz.
                    def filler(fc, n):
                        nc.tensor.matmul(
                            o_ps[(("A", "B")[fc % 2], fc // 2)][:, :n],
                            zz[:, :DC],
                            m_sb[0][:, :n],
                            start=False,
                            stop=False,
                        )

                    def emit_o(t):
                        # O(t) is emitted one slot late so the in-order PE
                        # queue never stalls on mul(t): by the time the PE
                        # reaches O(t), its P tile has long been ready.
                        for sub, _, h in heads:
                            for c2 in range(2):
                                nc.tensor.matmul(
                                    o_ps[(sub, c2)][:],
                                    v_sb[h][:, t * DC:(t + 1) * DC],
                                    p_t[t % 2][sub][:, c2 * 512:(c2 + 1) * 512],
                                    start=(t == 0),
                                    stop=(t == KT - 1),
                                )

                    p_t = [{}, {}]
                    for t in range(KT):
                        # O(t-1) first: it is dependency-free by now, runs
                        # while exp(t-1) still executes, and leaves nothing
                        # between S_B(t) and S_A(t+1) to delay the critical
                        # exp -> S -> exp chain.
                        if t >= 1:
                            emit_o(t - 1)
                        for sub, r0, _ in heads:
                            # two heads' S matmuls occupy PE row groups
                            # 0-63 / 64-127 (auto tile_position)
                            s_ps = ps_s.tile([128, QHALF], F32, tag=f"s{sub}")
                            for c2 in range(2):
                                nc.tensor.matmul(
                                    s_ps[:, c2 * 512:(c2 + 1) * 512],
                                    kT_sb[pr][r0:r0 + 64, t * 128:(t + 1) * 128],
                                    qT_sb[pr][r0:r0 + 64,
                                              q0 + c2 * 512:q0 + (c2 + 1) * 512],
                                    start=True,
                                    stop=True,
                                )
                            p0 = p_pool.tile([128, QHALF], BF16, tag=f"p{sub}")
                            nc.scalar.activation(p0[:], s_ps[:], EXP)
                            pm = pm_pool.tile([128, QHALF], BF16, tag=f"pm{sub}")
                            nc.vector.tensor_mul(
                                out=pm[:], in0=p0[:],
                                in1=m_sb[t][:, q0:q0 + QHALF],
                            )
                            p_t[t % 2][sub] = pm
                        if 1 <= t < KT - 1:
                            filler(t % 4, 256)
                            filler((t + 1) % 4, 256)
                        if pr == 0 and half == 0 and t <= 5:
                            # extra ramp fillers: the first slots run at
                            # input-DMA pace with the PE queue blocked, and
                            # a >3.4us PE lull here re-throttles the clock
                            # gate for the whole kernel
                            for f in range(3):
                                filler((t + f) % 4, 512)
                    emit_o(KT - 1)
                    # drain: copies split across ScalarE/VectorE so each
                    # O bank frees in ~0.7us and the next half's first O
                    # matmuls (WAR on these banks) never stall the PE queue
                    for sub, _, h in heads:
                        o_sb = osb_pool.tile([DC, QHALF], F32, tag="osb")
                        oeng = nc.sync if sub == "A" else nc.gpsimd
                        nc.scalar.copy(o_sb[:, 0:512], o_ps[(sub, 0)][:])
                        oeng.dma_start(out[h, :, q0:q0 + 512], o_sb[:, 0:512])
                        nc.vector.tensor_copy(o_sb[:, 512:1024], o_ps[(sub, 1)][:])
                        oeng.dma_start(out[h, :, q0 + 512:q0 + QHALF],
                                       o_sb[:, 512:1024])
    nc.compile()
    return nc


def _prep_in_maps(q, k, v, mask, s):
    import ml_dtypes

    bf16 = ml_dtypes.bfloat16
    # host prep: fold 1/scale into q; transpose to [d, seq]; pair-stack heads
    qh = (q / s).reshape(B * H, SQ, D).transpose(0, 2, 1)  # [32, 64, 2048]
    kh = k.reshape(B * H, SKV, D).transpose(0, 2, 1)
    vh = v.reshape(B * H, SKV, D)
    vA = np.concatenate(
        [np.ones((B * H, SKV, 1), dtype=np.float32), vh], axis=2
    ).astype(bf16)  # [32, 2048, 65], col 0 = ones
    # pre-arrange to the SBUF layout [head, 128, KT*DC] for contiguous DMA
    vA = np.ascontiguousarray(
        vA.reshape(B * H, KT, 128, DC).transpose(0, 2, 1, 3)
    ).reshape(B * H, 128, KT * DC)
    # exp of the transposed mask, shared across heads: exp(S+M) = exp(S)*exp(M)
    expM = np.exp(np.ascontiguousarray(mask.reshape(SQ, SKV).T)).astype(bf16)

    in_maps = []
    for c in range(NCORES):
        h0 = c * HPC
        qTc = np.ascontiguousarray(
            qh[h0:h0 + HPC].reshape(NPAIR, 128, SQ)
        ).astype(bf16)
        kTc = np.ascontiguousarray(
            kh[h0:h0 + HPC].reshape(NPAIR, 128, SKV)
        ).astype(bf16)
        vAc = np.ascontiguousarray(vA[h0:h0 + HPC])
        in_maps.append({"qT": qTc, "kT": kTc, "vA": vAc, "expM": expM})
    return in_maps


def kernel(q, k, v, mask, scale):
    from concourse.bass_utils import run_bass_kernel_spmd

    q = np.asarray(q, dtype=np.float32)
    k = np.asarray(k, dtype=np.float32)
    v = np.asarray(v, dtype=np.float32)
    mask = np.asarray(mask, dtype=np.float32)
    s = float(np.asarray(scale))

    in_maps = _prep_in_maps(q, k, v, mask, s)

    if "nc" not in _cached:
        _cached["nc"] = _build()
    res = run_bass_kernel_spmd(_cached["nc"], in_maps, list(range(NCORES)))

    # device emits unnormalized [head, 65, SQ]: row 0 = softmax denominator
    outs = []
    for c in range(NCORES):
        o = res.results[c]["out"]  # [HPC, DC, SQ] f32
        outs.append(o[:, 1:, :] / o[:, 0:1, :])
    full = np.concatenate(outs, axis=0)  # [32, 64, SQ]
    return np.ascontiguousarray(full.transpose(0, 2, 1)).reshape(B, H, SQ, D)


# revision 39
# speedup vs baseline: 1.0906x; 1.0032x over previous
"""Trainium2 Bass kernel for batched multi-head attention with additive mask.

Problem (full shapes): q,k,v [2,16,2048,64] f32, mask [1,1,2048,2048] f32,
scale scalar; out = softmax(q@k^T/scale + mask) @ v -> [2,16,2048,64].

Sharding: B*H = 32 heads split over 8 cores (4 heads/core), pure data
parallel, no collectives. The shared mask is replicated to every core.

Key idea vs the straightforward version: softmax(S + M) uses
exp(S + M) = exp(S) * exp(M), and M is shared across all heads. The host
precomputes expM = exp(M^T) once (bf16); the device then needs only
  P = exp(S) (ScalarE, PSUM->SBUF bf16)  *  expM tile (DVE bf16 2x mul)
instead of an f32 PSUM mask-add (1x DVE) + exp. The device also skips
normalization entirely: it emits O' = [denom | O]^T = [ones|V]'^T @ P^T
per head ([65, SQ] f32, denominator in row 0) and the host divides and
transposes. That removes the PE transposes, ScalarE copies and DVE
reciprocals of the normalize stage and frees PSUM banks.

Per-core device algorithm (per head pair, q-half, kv-tile):
  - S^T [128 kv, 1024 q] = kT.T @ qT, contraction d=64. The two heads of
    a pair sit in partitions 0-63 / 64-127 of pair-stacked qT/kT tiles,
    so their matmuls auto-derive PE tile_position (0,0)/(64,0) and run
    concurrently in the array (row tiling), recovering the half-array
    loss of the d=64 contraction.
  - P0 = exp(S^T): ScalarE PSUM -> SBUF bf16.
  - P = P0 * expM[t]: DVE bf16 tensor_tensor (2x mode).
  - O'^T [65, 1024] += V'[kv,65].T @ P^T, V' = [ones | V] bf16,
    accumulated f32 in PSUM over the 16 kv tiles.
  - DVE copy O' PSUM -> SBUF, DMA to DRAM [head, 65, SQ].
PE warm-up matmuls on a zero tile (no DMA dependency) keep the HAM clock
gate at 8/8 through the input-DMA prologue.
"""

import numpy as np

B, H, SQ, SKV, D = 2, 16, 2048, 2048, 64
NCORES = 8
HPC = (B * H) // NCORES  # heads per core = 4
NPAIR = HPC // 2  # head pairs per core = 2
KT = SKV // 128  # kv tiles = 16
QHALF = SQ // 2  # 1024
DC = D + 1  # 65 rows: [denom | O]

_cached = {}


def _build():
    from concourse import bacc
    import concourse.mybir as mybir
    import concourse.tile as tile

    F32 = mybir.dt.float32
    BF16 = mybir.dt.bfloat16
    EXP = mybir.ActivationFunctionType.Exp

    nc = bacc.Bacc("TRN2", target_bir_lowering=False, debug=False,
                   num_devices=NCORES)

    qT = nc.declare_dram_parameter("qT", [NPAIR, 128, SQ], BF16, isOutput=False)
    kT = nc.declare_dram_parameter("kT", [NPAIR, 128, SKV], BF16, isOutput=False)
    # vA is host-pre-arranged to the SBUF-resident layout [128, KT*DC] so
    # its DMA is fully contiguous (the naive [SKV, DC] layout needs 130-byte
    # strided segments whose descriptors occupy the DMA queue for ~12us and
    # starve the mask-tile loads behind them)
    vA = nc.declare_dram_parameter("vA", [HPC, 128, KT * DC], BF16, isOutput=False)
    expM = nc.declare_dram_parameter("expM", [SKV, SQ], BF16, isOutput=False)
    out = nc.declare_dram_parameter("out", [HPC, DC, SQ], F32, isOutput=True)

    with tile.TileContext(nc) as tc:
        with (
            tc.tile_pool(name="qk", bufs=1) as qk_pool,
            tc.tile_pool(name="vp", bufs=1) as v_pool,
            tc.tile_pool(name="m", bufs=1) as m_pool,
            tc.tile_pool(name="z", bufs=1) as z_pool,
            tc.tile_pool(name="p", bufs=3) as p_pool,
            tc.tile_pool(name="pm", bufs=3) as pm_pool,
            tc.tile_pool(name="osb", bufs=4) as osb_pool,
            tc.tile_pool(name="ps_s", bufs=1, space="PSUM") as ps_s,
            tc.tile_pool(name="ps_o", bufs=1, space="PSUM") as ps_o,
        ):
            # zero tile for PE warm-up: no DMA dependency, issues at t=0
            zz = z_pool.tile([128, 640], BF16, tag="z", name="zz")
            nc.vector.memset(zz[:], 0.0)

            # resident qT/kT, pair-stacked [128, seq]; pair 0 first so the
            # main loop can start as early as possible
            # pair-0 q/k split so the first S matmuls' exact columns land
            # first: k cols 0:256 (64KB) on scalar, q cols 0:1024 on sync;
            # the rest follows behind on the same queues
            qT_sb = [None] * NPAIR
            kT_sb = [None] * NPAIR
            qt = qk_pool.tile([128, SQ], BF16, tag="q0", name="q0")
            nc.sync.dma_start(qt[:, 0:QHALF], qT[0][:, 0:QHALF])
            qT_sb[0] = qt
            kt = qk_pool.tile([128, SKV], BF16, tag="k0", name="k0")
            nc.scalar.dma_start(kt[:, 0:256], kT[0][:, 0:256])
            nc.sync.dma_start(kt[:, 256:], kT[0][:, 256:])
            kT_sb[0] = kt

            # PE warm-up part 1: back-to-back matmuls on the zero tile keep
            # the PE busy while the first input DMAs land.
            wu_ps = ps_s.tile([128, QHALF], F32, tag="sA", name="wu")
            for w in range(8):
                nc.tensor.matmul(
                    wu_ps[:, :512], zz[:, :128], zz[:, 128:640],
                    start=True, stop=True,
                )

            # Input DMAs are spread across engine DGE queues (each engine
            # owns one HW queue, ~180 GB/s each): q/k on sync, V + late
            # masks on gpsimd (idle all kernel), early masks on vector,
            # last masks on scalar. One serial queue would take ~20us+
            # before the first O matmul can run.
            # Each mask tile is DMA'd in two column halves: the first
            # q-half of compute reads only columns 0:1024, so the hi
            # halves (4.2MB) are deferred off the ramp's critical path
            # (subtile deps let each mul wait on just its half).
            m_sb = [None] * KT

            def m_dma(t, eng, half):
                if m_sb[t] is None:
                    m_sb[t] = m_pool.tile(
                        [128, SQ], BF16, tag=f"m{t}", name=f"m{t}"
                    )
                c = half * QHALF
                eng.dma_start(
                    m_sb[t][:, c:c + QHALF],
                    expM[t * 128:(t + 1) * 128, c:c + QHALF],
                )

            # scalar gets only two issues: its FIFO must stay clear for exps
            m_dma(0, nc.scalar, 0)
            m_dma(1, nc.sync, 0)
            m_dma(2, nc.scalar, 0)
            m_dma(3, nc.sync, 0)

            v_sb = []
            for h in range(HPC):
                vt = v_pool.tile([128, KT * DC], BF16, tag=f"v{h}", name=f"v{h}")
                nc.gpsimd.dma_start(vt[:], vA[h])
                v_sb.append(vt)

            # PE warm-up part 2: gated on the k0 DMA, so it runs gap-free
            # right before the first real S matmuls. The HAM clock gate
            # only reaches 8/8 after ~3.4us of *continuous* PE busy; the
            # main loop's dependency micro-gaps never re-warm it, so the
            # warm state must be established here and never dropped.
            wu2_ps = ps_s.tile([128, QHALF], F32, tag="sB", name="wu2")
            for w in range(8):
                nc.tensor.matmul(
                    wu2_ps[:, :512], kT_sb[0][:, 0:128], zz[:, 128:640],
                    start=True, stop=True,
                )

            for t in range(4, KT):
                m_dma(t, nc.sync if t % 2 else nc.gpsimd, 0)
            # q0 hi half: not needed until the second q-half
            nc.gpsimd.dma_start(qT_sb[0][:, QHALF:], qT[0][:, QHALF:])
            # hi mask halves: not needed until the second q-half (~50us)
            for t in range(KT):
                m_dma(t, nc.gpsimd, 1)

            # pair-1 q/k: not needed until ~90us in
            qt = qk_pool.tile([128, SQ], BF16, tag="q1", name="q1")
            nc.gpsimd.dma_start(qt[:], qT[1])
            qT_sb[1] = qt
            kt = qk_pool.tile([128, SKV], BF16, tag="k1", name="k1")
            nc.gpsimd.dma_start(kt[:], kT[1])
            kT_sb[1] = kt

            for pr in range(NPAIR):
                heads = (("A", 0, 2 * pr), ("B", 64, 2 * pr + 1))
                for half in range(2):
                    q0 = half * QHALF
                    o_ps = {}
                    for sub, _, _ in heads:
                        for c2 in range(2):
                            o_ps[(sub, c2)] = ps_o.tile(
                                [DC, 512], F32, tag=f"o{sub}{c2}",
                                name=f"o{sub}{c2}",
                            )
                    # zero-weight filler matmuls: accumulate +0 into the
                    # live O tiles (start=False -> pure PE busy-work, no
                    # extra PSUM). They pad the PE stream just enough that
                    # the HAM clock gate sees near-continuous activity and
                    # stays 8/8; without them HAM re-throttles the whole
                    # kernel to 1.2 GHz.
                    def filler(fc, n):
                        nc.tensor.matmul(
                            o_ps[(("A", "B")[fc % 2], fc // 2)][:, :n],
                            zz[:, :DC],
                            m_sb[0][:, :n],
                            start=False,
                            stop=False,
                        )

                    def emit_o(t):
                        # O(t) is emitted two slots late: the in-order PE
                        # queue never stalls on mul(t), and at each half
                        # boundary the drain copies get ~2 slots of slack
                        # before O(0) of the next half (WAR on the drained
                        # banks) enters the queue.
                        for sub, _, h in heads:
                            for c2 in range(2):
                                nc.tensor.matmul(
                                    o_ps[(sub, c2)][:],
                                    v_sb[h][:, t * DC:(t + 1) * DC],
                                    p_t[t % 3][sub][:, c2 * 512:(c2 + 1) * 512],
                                    start=(t == 0),
                                    stop=(t == KT - 1),
                                )

                    p_t = [{}, {}, {}]
                    for t in range(KT):
                        # O(t-2) first: it is dependency-free by now, runs
                        # while exp(t-1) still executes, and leaves nothing
                        # between S_B(t) and S_A(t+1) to delay the critical
                        # exp -> S -> exp chain.
                        if t >= 2:
                            emit_o(t - 2)
                        for sub, r0, _ in heads:
                            # two heads' S matmuls occupy PE row groups
                            # 0-63 / 64-127 (auto tile_position)
                            s_ps = ps_s.tile([128, QHALF], F32, tag=f"s{sub}")
                            for c2 in range(2):
                                nc.tensor.matmul(
                                    s_ps[:, c2 * 512:(c2 + 1) * 512],
                                    kT_sb[pr][r0:r0 + 64, t * 128:(t + 1) * 128],
                                    qT_sb[pr][r0:r0 + 64,
                                              q0 + c2 * 512:q0 + (c2 + 1) * 512],
                                    start=True,
                                    stop=True,
                                )
                            p0 = p_pool.tile([128, QHALF], BF16, tag=f"p{sub}")
                            nc.scalar.activation(p0[:], s_ps[:], EXP)
                            pm = pm_pool.tile([128, QHALF], BF16, tag=f"pm{sub}")
                            nc.vector.tensor_mul(
                                out=pm[:], in0=p0[:],
                                in1=m_sb[t][:, q0:q0 + QHALF],
                            )
                            p_t[t % 3][sub] = pm
                        if 1 <= t < KT - 1:
                            filler(t % 4, 256)
                            filler((t + 1) % 4, 256)
                        if pr == 0 and half == 0 and t <= 5:
                            # extra ramp fillers: the first slots run at
                            # input-DMA pace with the PE queue blocked, and
                            # a >3.4us PE lull here re-throttles the clock
                            # gate for the whole kernel
                            for f in range(3):
                                filler((t + f) % 4, 512)
                    emit_o(KT - 2)
                    emit_o(KT - 1)
                    # drain: copies split across ScalarE/VectorE so each
                    # O bank frees in ~0.7us and the next half's first O
                    # matmuls (WAR on these banks) never stall the PE queue
                    for sub, _, h in heads:
                        o_sb = osb_pool.tile([DC, QHALF], F32, tag="osb")
                        oeng = nc.sync if sub == "A" else nc.gpsimd
                        nc.scalar.copy(o_sb[:, 0:512], o_ps[(sub, 0)][:])
                        oeng.dma_start(out[h, :, q0:q0 + 512], o_sb[:, 0:512])
                        nc.vector.tensor_copy(o_sb[:, 512:1024], o_ps[(sub, 1)][:])
                        oeng.dma_start(out[h, :, q0 + 512:q0 + QHALF],
                                       o_sb[:, 512:1024])
    nc.compile()
    return nc


def _prep_in_maps(q, k, v, mask, s):
    import ml_dtypes

    bf16 = ml_dtypes.bfloat16
    # host prep: fold 1/scale into q; transpose to [d, seq]; pair-stack heads
    qh = (q / s).reshape(B * H, SQ, D).transpose(0, 2, 1)  # [32, 64, 2048]
    kh = k.reshape(B * H, SKV, D).transpose(0, 2, 1)
    vh = v.reshape(B * H, SKV, D)
    vA = np.concatenate(
        [np.ones((B * H, SKV, 1), dtype=np.float32), vh], axis=2
    ).astype(bf16)  # [32, 2048, 65], col 0 = ones
    # pre-arrange to the SBUF layout [head, 128, KT*DC] for contiguous DMA
    vA = np.ascontiguousarray(
        vA.reshape(B * H, KT, 128, DC).transpose(0, 2, 1, 3)
    ).reshape(B * H, 128, KT * DC)
    # exp of the transposed mask, shared across heads: exp(S+M) = exp(S)*exp(M)
    expM = np.exp(np.ascontiguousarray(mask.reshape(SQ, SKV).T)).astype(bf16)

    in_maps = []
    for c in range(NCORES):
        h0 = c * HPC
        qTc = np.ascontiguousarray(
            qh[h0:h0 + HPC].reshape(NPAIR, 128, SQ)
        ).astype(bf16)
        kTc = np.ascontiguousarray(
            kh[h0:h0 + HPC].reshape(NPAIR, 128, SKV)
        ).astype(bf16)
        vAc = np.ascontiguousarray(vA[h0:h0 + HPC])
        in_maps.append({"qT": qTc, "kT": kTc, "vA": vAc, "expM": expM})
    return in_maps


def kernel(q, k, v, mask, scale):
    from concourse.bass_utils import run_bass_kernel_spmd

    q = np.asarray(q, dtype=np.float32)
    k = np.asarray(k, dtype=np.float32)
    v = np.asarray(v, dtype=np.float32)
    mask = np.asarray(mask, dtype=np.float32)
    s = float(np.asarray(scale))

    in_maps = _prep_in_maps(q, k, v, mask, s)

    if "nc" not in _cached:
        _cached["nc"] = _build()
    res = run_bass_kernel_spmd(_cached["nc"], in_maps, list(range(NCORES)))

    # device emits unnormalized [head, 65, SQ]: row 0 = softmax denominator
    outs = []
    for c in range(NCORES):
        o = res.results[c]["out"]  # [HPC, DC, SQ] f32
        outs.append(o[:, 1:, :] / o[:, 0:1, :])
    full = np.concatenate(outs, axis=0)  # [32, 64, SQ]
    return np.ascontiguousarray(full.transpose(0, 2, 1)).reshape(B, H, SQ, D)


# revision 41
# speedup vs baseline: 1.1259x; 1.0324x over previous
"""Trainium2 Bass kernel for batched multi-head attention with additive mask.

Problem (full shapes): q,k,v [2,16,2048,64] f32, mask [1,1,2048,2048] f32,
scale scalar; out = softmax(q@k^T/scale + mask) @ v -> [2,16,2048,64].

Sharding: B*H = 32 heads split over 8 cores (4 heads/core), pure data
parallel, no collectives. The shared mask is replicated to every core.

Key idea vs the straightforward version: softmax(S + M) uses
exp(S + M) = exp(S) * exp(M), and M is shared across all heads. The host
precomputes expM = exp(M^T) once (bf16); the device then needs only
  P = exp(S) (ScalarE, PSUM->SBUF bf16)  *  expM tile (DVE bf16 2x mul)
instead of an f32 PSUM mask-add (1x DVE) + exp. The device also skips
normalization entirely: it emits O' = [denom | O]^T = [ones|V]'^T @ P^T
per head ([65, SQ] f32, denominator in row 0) and the host divides and
transposes. That removes the PE transposes, ScalarE copies and DVE
reciprocals of the normalize stage and frees PSUM banks.

Per-core device algorithm (per head pair, q-half, kv-tile):
  - S^T [128 kv, 1024 q] = kT.T @ qT, contraction d=64. The two heads of
    a pair sit in partitions 0-63 / 64-127 of pair-stacked qT/kT tiles,
    so their matmuls auto-derive PE tile_position (0,0)/(64,0) and run
    concurrently in the array (row tiling), recovering the half-array
    loss of the d=64 contraction.
  - P0 = exp(S^T): ScalarE PSUM -> SBUF bf16.
  - P = P0 * expM[t]: DVE bf16 tensor_tensor (2x mode).
  - O'^T [65, 1024] += V'[kv,65].T @ P^T, V' = [ones | V] bf16,
    accumulated f32 in PSUM over the 16 kv tiles.
  - DVE copy O' PSUM -> SBUF, DMA to DRAM [head, 65, SQ].
PE warm-up matmuls on a zero tile (no DMA dependency) keep the HAM clock
gate at 8/8 through the input-DMA prologue.
"""

import numpy as np

B, H, SQ, SKV, D = 2, 16, 2048, 2048, 64
NCORES = 8
HPC = (B * H) // NCORES  # heads per core = 4
NPAIR = HPC // 2  # head pairs per core = 2
KT = SKV // 128  # kv tiles = 16
QHALF = SQ // 2  # 1024
DC = D + 1  # 65 rows: [denom | O]

_cached = {}


def _build():
    from concourse import bacc
    import concourse.mybir as mybir
    import concourse.tile as tile

    F32 = mybir.dt.float32
    BF16 = mybir.dt.bfloat16
    EXP = mybir.ActivationFunctionType.Exp

    nc = bacc.Bacc("TRN2", target_bir_lowering=False, debug=False,
                   num_devices=NCORES)

    qT = nc.declare_dram_parameter("qT", [NPAIR, 128, SQ], BF16, isOutput=False)
    kT = nc.declare_dram_parameter("kT", [NPAIR, 128, SKV], BF16, isOutput=False)
    # vA is host-pre-arranged to the SBUF-resident layout [128, KT*DC] so
    # its DMA is fully contiguous (the naive [SKV, DC] layout needs 130-byte
    # strided segments whose descriptors occupy the DMA queue for ~12us and
    # starve the mask-tile loads behind them)
    vA = nc.declare_dram_parameter("vA", [HPC, 128, KT * DC], BF16, isOutput=False)
    expM = nc.declare_dram_parameter("expM", [SKV, SQ], BF16, isOutput=False)
    out = nc.declare_dram_parameter("out", [HPC, DC, SQ], F32, isOutput=True)

    with tile.TileContext(nc) as tc:
        with (
            tc.tile_pool(name="qk", bufs=1) as qk_pool,
            tc.tile_pool(name="vp", bufs=1) as v_pool,
            tc.tile_pool(name="m", bufs=1) as m_pool,
            tc.tile_pool(name="z", bufs=1) as z_pool,
            tc.tile_pool(name="p", bufs=4) as p_pool,
            tc.tile_pool(name="pm", bufs=4) as pm_pool,
            tc.tile_pool(name="osb", bufs=4) as osb_pool,
            tc.tile_pool(name="ps_s", bufs=3, space="PSUM") as ps_s,
            tc.tile_pool(name="ps_o", bufs=1, space="PSUM") as ps_o,
        ):
            # zero tile for PE warm-up: no DMA dependency, issues at t=0
            zz = z_pool.tile([128, 640], BF16, tag="z", name="zz")
            nc.vector.memset(zz[:], 0.0)

            # resident qT/kT, pair-stacked [128, seq]; pair 0 first so the
            # main loop can start as early as possible
            # pair-0 q/k split so the first S matmuls' exact columns land
            # first: k cols 0:256 (64KB) on scalar, q cols 0:1024 on sync;
            # the rest follows behind on the same queues
            qT_sb = [None] * NPAIR
            kT_sb = [None] * NPAIR
            qt = qk_pool.tile([128, SQ], BF16, tag="q0", name="q0")
            nc.sync.dma_start(qt[:, 0:QHALF], qT[0][:, 0:QHALF])
            qT_sb[0] = qt
            kt = qk_pool.tile([128, SKV], BF16, tag="k0", name="k0")
            nc.scalar.dma_start(kt[:, 0:256], kT[0][:, 0:256])
            nc.sync.dma_start(kt[:, 256:], kT[0][:, 256:])
            kT_sb[0] = kt

            # PE warm-up part 1: back-to-back matmuls on the zero tile keep
            # the PE busy while the first input DMAs land.
            wu_ps = ps_s.tile([128, QHALF], F32, tag="s", name="wu")
            for w in range(8):
                nc.tensor.matmul(
                    wu_ps[:, :512], zz[:, :128], zz[:, 128:640],
                    start=True, stop=True,
                )

            # Input DMAs are spread across engine DGE queues (each engine
            # owns one HW queue, ~180 GB/s each): q/k on sync, V + late
            # masks on gpsimd (idle all kernel), early masks on vector,
            # last masks on scalar. One serial queue would take ~20us+
            # before the first O matmul can run.
            # Each mask tile is DMA'd in two column halves: the first
            # q-half of compute reads only columns 0:1024, so the hi
            # halves (4.2MB) are deferred off the ramp's critical path
            # (subtile deps let each mul wait on just its half).
            m_sb = [None] * KT

            def m_dma(t, eng, half):
                if m_sb[t] is None:
                    m_sb[t] = m_pool.tile(
                        [128, SQ], BF16, tag=f"m{t}", name=f"m{t}"
                    )
                c = half * QHALF
                eng.dma_start(
                    m_sb[t][:, c:c + QHALF],
                    expM[t * 128:(t + 1) * 128, c:c + QHALF],
                )

            # scalar gets only two issues: its FIFO must stay clear for exps
            m_dma(0, nc.scalar, 0)
            m_dma(1, nc.sync, 0)
            m_dma(2, nc.scalar, 0)
            m_dma(3, nc.sync, 0)

            v_sb = []
            for h in range(HPC):
                vt = v_pool.tile([128, KT * DC], BF16, tag=f"v{h}", name=f"v{h}")
                nc.gpsimd.dma_start(vt[:], vA[h])
                v_sb.append(vt)

            # PE warm-up part 2: gated on the k0 DMA, so it runs gap-free
            # right before the first real S matmuls. The HAM clock gate
            # only reaches 8/8 after ~3.4us of *continuous* PE busy; the
            # main loop's dependency micro-gaps never re-warm it, so the
            # warm state must be established here and never dropped.
            wu2_ps = ps_s.tile([128, QHALF], F32, tag="s", name="wu2")
            for w in range(8):
                nc.tensor.matmul(
                    wu2_ps[:, :512], kT_sb[0][:, 0:128], zz[:, 128:640],
                    start=True, stop=True,
                )

            for t in range(4, KT):
                m_dma(t, nc.sync if t % 2 else nc.gpsimd, 0)
            # q0 hi half: not needed until the second q-half
            nc.gpsimd.dma_start(qT_sb[0][:, QHALF:], qT[0][:, QHALF:])
            # hi mask halves: not needed until the second q-half (~50us)
            for t in range(KT):
                m_dma(t, nc.gpsimd, 1)

            # pair-1 q/k: not needed until ~90us in
            qt = qk_pool.tile([128, SQ], BF16, tag="q1", name="q1")
            nc.gpsimd.dma_start(qt[:], qT[1])
            qT_sb[1] = qt
            kt = qk_pool.tile([128, SKV], BF16, tag="k1", name="k1")
            nc.gpsimd.dma_start(kt[:], kT[1])
            kT_sb[1] = kt

            for h in range(HPC):
                pr, r0 = h // 2, (h % 2) * 64
                for half in range(2):
                    q0 = half * QHALF
                    o_ps = {
                        c2: ps_o.tile([DC, 512], F32, tag=f"o{c2}",
                                      name=f"o{c2}")
                        for c2 in range(2)
                    }

                    # zero-weight filler matmuls: accumulate +0 into the
                    # live O tiles (start=False -> pure PE busy-work, no
                    # extra PSUM). Sized to just top the PE off near the
                    # Scalar slot period so the HAM clock gate stays 8/8
                    # without making the PE the limiter.
                    def filler(fc, n):
                        nc.tensor.matmul(
                            o_ps[fc % 2][:, :n],
                            zz[:, :DC],
                            m_sb[0][:, :n],
                            start=False,
                            stop=False,
                        )

                    def emit_o(t):
                        # O(t) is emitted two slots late: the in-order PE
                        # queue never stalls on mul(t), and at boundaries
                        # the drain copies get ~2 slots of slack before
                        # O(0) of the next half (WAR on the drained banks)
                        # enters the queue.
                        for c2 in range(2):
                            nc.tensor.matmul(
                                o_ps[c2][:],
                                v_sb[h][:, t * DC:(t + 1) * DC],
                                p_t[t % 3][:, c2 * 512:(c2 + 1) * 512],
                                start=(t == 0),
                                stop=(t == KT - 1),
                            )

                    p_t = [None, None, None]
                    for t in range(KT):
                        if t >= 2:
                            emit_o(t - 2)
                        # S psum rotates through a shared 3-buffer pool, so
                        # S(t) only WARs exp(t-3): the exp stream runs
                        # back-to-back with no S-chain bubble, including
                        # across (head, half) boundaries.
                        s_ps = ps_s.tile([128, QHALF], F32, tag="s")
                        for c2 in range(2):
                            nc.tensor.matmul(
                                s_ps[:, c2 * 512:(c2 + 1) * 512],
                                kT_sb[pr][r0:r0 + 64, t * 128:(t + 1) * 128],
                                qT_sb[pr][r0:r0 + 64,
                                          q0 + c2 * 512:q0 + (c2 + 1) * 512],
                                start=True,
                                stop=True,
                            )
                        p0 = p_pool.tile([128, QHALF], BF16, tag="p")
                        nc.scalar.activation(p0[:], s_ps[:], EXP)
                        pm = pm_pool.tile([128, QHALF], BF16, tag="pm")
                        nc.vector.tensor_mul(
                            out=pm[:], in0=p0[:],
                            in1=m_sb[t][:, q0:q0 + QHALF],
                        )
                        p_t[t % 3] = pm
                        if 1 <= t < KT - 1:
                            filler(t, 128)
                        if h == 0 and half == 0 and t <= 5:
                            # extra ramp fillers: the first slots run at
                            # input-DMA pace with the PE queue blocked, and
                            # a >3.4us PE lull here re-throttles the clock
                            # gate for the whole kernel
                            for f in range(3):
                                filler(t + f, 512)
                    emit_o(KT - 2)
                    emit_o(KT - 1)
                    # drain on DVE only: the Scalar FIFO stays pure exps
                    o_sb = osb_pool.tile([DC, QHALF], F32, tag="osb")
                    oeng = nc.sync if h % 2 == 0 else nc.gpsimd
                    for c2 in range(2):
                        nc.vector.tensor_copy(
                            o_sb[:, c2 * 512:(c2 + 1) * 512], o_ps[c2][:]
                        )
                        oeng.dma_start(
                            out[h, :, q0 + c2 * 512:q0 + (c2 + 1) * 512],
                            o_sb[:, c2 * 512:(c2 + 1) * 512],
                        )
    nc.compile()
    return nc


def _prep_in_maps(q, k, v, mask, s):
    import ml_dtypes

    bf16 = ml_dtypes.bfloat16
    # host prep: fold 1/scale into q; transpose to [d, seq]; pair-stack heads
    qh = (q / s).reshape(B * H, SQ, D).transpose(0, 2, 1)  # [32, 64, 2048]
    kh = k.reshape(B * H, SKV, D).transpose(0, 2, 1)
    vh = v.reshape(B * H, SKV, D)
    vA = np.concatenate(
        [np.ones((B * H, SKV, 1), dtype=np.float32), vh], axis=2
    ).astype(bf16)  # [32, 2048, 65], col 0 = ones
    # pre-arrange to the SBUF layout [head, 128, KT*DC] for contiguous DMA
    vA = np.ascontiguousarray(
        vA.reshape(B * H, KT, 128, DC).transpose(0, 2, 1, 3)
    ).reshape(B * H, 128, KT * DC)
    # exp of the transposed mask, shared across heads: exp(S+M) = exp(S)*exp(M)
    expM = np.exp(np.ascontiguousarray(mask.reshape(SQ, SKV).T)).astype(bf16)

    in_maps = []
    for c in range(NCORES):
        h0 = c * HPC
        qTc = np.ascontiguousarray(
            qh[h0:h0 + HPC].reshape(NPAIR, 128, SQ)
        ).astype(bf16)
        kTc = np.ascontiguousarray(
            kh[h0:h0 + HPC].reshape(NPAIR, 128, SKV)
        ).astype(bf16)
        vAc = np.ascontiguousarray(vA[h0:h0 + HPC])
        in_maps.append({"qT": qTc, "kT": kTc, "vA": vAc, "expM": expM})
    return in_maps


def kernel(q, k, v, mask, scale):
    from concourse.bass_utils import run_bass_kernel_spmd

    q = np.asarray(q, dtype=np.float32)
    k = np.asarray(k, dtype=np.float32)
    v = np.asarray(v, dtype=np.float32)
    mask = np.asarray(mask, dtype=np.float32)
    s = float(np.asarray(scale))

    in_maps = _prep_in_maps(q, k, v, mask, s)

    if "nc" not in _cached:
        _cached["nc"] = _build()
    res = run_bass_kernel_spmd(_cached["nc"], in_maps, list(range(NCORES)))

    # device emits unnormalized [head, 65, SQ]: row 0 = softmax denominator
    outs = []
    for c in range(NCORES):
        o = res.results[c]["out"]  # [HPC, DC, SQ] f32
        outs.append(o[:, 1:, :] / o[:, 0:1, :])
    full = np.concatenate(outs, axis=0)  # [32, 64, SQ]
    return np.ascontiguousarray(full.transpose(0, 2, 1)).reshape(B, H, SQ, D)
